# revision 31
# baseline (speedup 1.0000x reference)
"""Chunked DeltaNet layer on 8 TRN2 NeuronCores — v2.

Sharding: core c -> batch b = c//2, head-group hh = c%2 (heads hh*4..hh*4+3).
Each core: q/k/v projections + causal conv + normalization for its 4 heads,
chunked (WY-form) delta rule over L=2048 in 16 chunks of 128, RMS norm,
partial out-projection [2048, 1024]. Host sums the two partials per batch.

Changes vs v1 (601us baseline):
  - all per-chunk scalar chains hoisted to per-lt [4,512] chains computed
    once per 512 tokens (k-norm rk, beta' rkb, eps*||q||^2), off the PE
    critical path; chunk phase needs only [4,128] transposes of them.
  - q-normalization eliminated algebraically: rq cancels in the output RMS
    norm; ro = rsqrt(mean(po^2)/HD + eps*||q||^2) (exact to ~1e-5).
  - beta' uses kn2n -> 1 (error ~6e-6) and bp = beta/(1+beta) =
    0.25 + 0.25*tanh(z/2 + ln2/2) — tanh shares the silu act table, so the
    Scalar engine cycles only two tables: silu/tanh per lt, sqrt per chunk.
  - v-silu applied per-lt in [feat,time] layout (AF.Silu, 4 ops per lt).
  - per-chunk RMS norm batched across heads (one sqrt + one approx-recip
    per chunk); state-update matmuls emitted before the output matmuls so
    the serial state chain clears early.
  - proj/conv for lt+1 emitted in halves inside lt's chunk loop to keep
    the PE p-state ramp warm across lt boundaries.
"""

import contextlib

import ml_dtypes
import numpy as np

import concourse.bass as bass
import concourse.mybir as mybir
import concourse.tile as tile
from concourse import bacc

F32 = mybir.dt.float32
BF16 = mybir.dt.bfloat16
AF = mybir.ActivationFunctionType
ALU = mybir.AluOpType

B, L, D, H, HD, CONV = 4, 2048, 1024, 8, 128, 4
ETA, EPS = 1.0, 1e-6
C = 128
NCH = L // C
NLT = 4
LT = 512
HL = 4
KS = D // 128
SIG = ("k", "q", "v")


def build_nc():
    nc = bacc.Bacc("TRN2", target_bir_lowering=False, debug=False)

    xt_d = nc.dram_tensor("xt", [KS, 128, L], BF16, kind="ExternalInput").ap()
    wq_d = nc.dram_tensor("wq", [KS, 128, 512], BF16, kind="ExternalInput").ap()
    wk_d = nc.dram_tensor("wk", [KS, 128, 512], BF16, kind="ExternalInput").ap()
    wv_d = nc.dram_tensor("wv", [KS, 128, 512], BF16, kind="ExternalInput").ap()
    wb_d = nc.dram_tensor("wb", [KS, 128, 4], BF16, kind="ExternalInput").ap()
    wo_d = nc.dram_tensor("wo", [4, 128, 1024], BF16, kind="ExternalInput").ap()
    cd_d = nc.dram_tensor("cd", [12, CONV, 128, 128], BF16, kind="ExternalInput").ap()
    mk_d = nc.dram_tensor("mk", [2, 128, 128], F32, kind="ExternalInput").ap()
    oh_d = nc.dram_tensor("oh", [8, 128, 8], BF16, kind="ExternalInput").ap()
    id16_d = nc.dram_tensor("id16", [128, 128], BF16, kind="ExternalInput").ap()
    id32_d = nc.dram_tensor("id32", [128, 128], F32, kind="ExternalInput").ap()
    out_d = nc.dram_tensor("out", [L, D], F32, kind="ExternalOutput").ap()

    with tile.TileContext(nc) as tc, contextlib.ExitStack() as ctx:
        consts = ctx.enter_context(tc.tile_pool(name="consts", bufs=1))
        persist = ctx.enter_context(tc.tile_pool(name="persist", bufs=1))
        btp = ctx.enter_context(tc.tile_pool(name="btp", bufs=2))
        xtp = ctx.enter_context(tc.tile_pool(name="xtp", bufs=2))
        bchp = ctx.enter_context(tc.tile_pool(name="bchp", bufs=3))
        projp = ctx.enter_context(tc.tile_pool(name="projp", bufs=2))
        rawp = ctx.enter_context(tc.tile_pool(name="rawp", bufs=2))
        sqp = ctx.enter_context(tc.tile_pool(name="sqp", bufs=2))
        normp = ctx.enter_context(tc.tile_pool(name="normp", bufs=2))
        chainp = ctx.enter_context(tc.tile_pool(name="chainp", bufs=5))
        scalp = ctx.enter_context(tc.tile_pool(name="scalp", bufs=4))
        outp = ctx.enter_context(tc.tile_pool(name="outp", bufs=2))
        ps_big = ctx.enter_context(tc.tile_pool(name="ps_big", bufs=2, space="PSUM"))
        ps_t = ctx.enter_context(tc.tile_pool(name="ps_t", bufs=3, space="PSUM"))
        ps_mm = ctx.enter_context(tc.tile_pool(name="ps_mm", bufs=3, space="PSUM"))

        # ---- constants ----
        ws = {}
        for name, d in (("q", wq_d), ("k", wk_d), ("v", wv_d)):
            w = consts.tile([128, KS, 512], BF16, name=f"w{name}")
            for i in range(KS):
                nc.sync.dma_start(out=w[:, i, :], in_=d[i])
            ws[name] = w
        wb = consts.tile([128, KS, 4], BF16)
        for i in range(KS):
            nc.sync.dma_start(out=wb[:, i, :], in_=wb_d[i])
        wo = consts.tile([128, 4, 1024], BF16)
        for i in range(4):
            nc.sync.dma_start(out=wo[:, i, :], in_=wo_d[i])
        cd = consts.tile([128, 12, CONV, 128], BF16)
        for n_ in range(12):
            for j_ in range(CONV):
                nc.sync.dma_start(out=cd[:, n_, j_, :], in_=cd_d[n_, j_])
        mk = consts.tile([128, 2, 128], F32)
        for n_ in range(2):
            nc.sync.dma_start(out=mk[:, n_, :], in_=mk_d[n_])
        oh8 = consts.tile([128, 8, 8], BF16)
        for n_ in range(8):
            nc.sync.dma_start(out=oh8[:, n_, :], in_=oh_d[n_])
        id16 = consts.tile([128, 128], BF16)
        nc.sync.dma_start(out=id16, in_=id16_d)
        id32 = consts.tile([128, 128], F32)
        nc.sync.dma_start(out=id32, in_=id32_d)

        # ---- persistent ----
        # per-head column-major scale rows: rk, rkb=rk*beta/(1+beta), eps*||q||^2
        rk_cm = persist.tile([4, L], F32)
        rkb_cm = persist.tile([4, L], F32)
        qn2e_cm = persist.tile([4, L], F32)
        s16 = persist.tile([128, HL, 128], BF16)
        s32 = persist.tile([128, HL, 128], F32)
        ot = persist.tile([128, HL, L], BF16)
        nc.vector.memset(s16, 0.0)
        nc.vector.memset(s32, 0.0)
        lnb = persist.tile([4, 1], F32)
        nc.vector.memset(lnb, 0.34657359)  # ln(2)/2

        prev_proj = {}
        bt_tiles = {}

        xtl_cur = {}
        raw_cur = {}
        bp_cur = {}

        def emit_projconv(lt, part):
            """PE projections + conv (diag matmuls) for 2 heads; beta chain."""
            hs = (0, 1) if part == 0 else (2, 3)
            if part == 0:
                tsl = bass.ds(lt * LT, LT)
                xtl = xtp.tile([128, KS, LT], BF16, name="xtl", tag="xtl")
                for i in range(KS):
                    nc.sync.dma_start(out=xtl[:, i, :], in_=xt_d[i, :, tsl])
                xtl_cur[lt] = xtl
                raw_cur[lt] = {}
            xtl = xtl_cur[lt]
            raw = raw_cur[lt]
            for h in hs:
                for si, s in enumerate(SIG):
                    psj = ps_big.tile([128, LT], F32, name="psproj", tag="big")
                    for i in range(KS):
                        nc.tensor.matmul(
                            psj, ws[s][:, i, h * 128:(h + 1) * 128], xtl[:, i, :],
                            start=(i == 0), stop=(i == KS - 1))
                    pt = projp.tile([128, LT + 4], BF16, name="pt", tag=f"pj{s}{h}")
                    if lt == 0:
                        nc.scalar.memzero(pt[:, 0:4])
                    else:
                        nc.scalar.copy(pt[:, 0:3], prev_proj[(s, h)][:, LT:LT + 3])
                    nc.scalar.copy(pt[:, 3:LT + 3], psj)
                    prev_proj[(s, h)] = pt
                    # conv: 4 shifted diagonal matmuls on PE
                    n = si * HL + h
                    pc = ps_big.tile([128, LT], F32, name="psconv", tag="big")
                    for j in range(CONV):
                        nc.tensor.matmul(pc, cd[:, n, j, :], pt[:, j:LT + j],
                                         start=(j == 0), stop=(j == CONV - 1))
                    r = rawp.tile([128, LT], BF16, name="raw", tag=f"rw{s}{h}")
                    nc.scalar.copy(r, pc)
                    raw[(s, h)] = r
            if part == 0:
                # bp = beta/(1+beta) = 1/(2+e^-z) = 0.5*sigmoid(z+ln2)
                #    = 0.25 + 0.25*tanh(0.5*z + ln2/2)   (no reciprocal needed)
                psb = ps_big.tile([4, LT], F32, name="psbeta", tag="big")
                for i in range(KS):
                    nc.tensor.matmul(psb, wb[:, i, :], xtl[:, i, :],
                                     start=(i == 0), stop=(i == KS - 1))
                th = bchp.tile([4, LT], F32, name="th", tag="bchain")
                nc.scalar.activation(th, psb, AF.Tanh, scale=0.5, bias=lnb)
                bp = bchp.tile([4, LT], F32, name="bp", tag="bchain")
                nc.vector.tensor_scalar(bp, th, 0.25, 0.25, op0=ALU.mult, op1=ALU.add)
                bp_cur[lt] = bp
            return raw

        def emit_norms(lt):
            """silu(v), k/q norms, rk/rkb/qn2e chains, bt transposes for lt."""
            raw, bp = raw_cur[lt], bp_cur[lt]
            tsl = bass.ds(lt * LT, LT)
            # silu'd v per head (feat-major, elementwise so layout-free)
            for h in range(HL):
                rv = rawp.tile([128, LT], BF16, name="rsv", tag=f"rsv{h}")
                nc.scalar.activation(rv, raw[("v", h)], AF.Silu)
                raw[("v", h)] = rv
            # ||k||^2 and ||q||^2 per (head, t) via squares + one-hot matmuls
            psnk = ps_big.tile([4, LT], F32, name="psnk", tag="big")
            psnq = ps_big.tile([4, LT], F32, name="psnq", tag="big")
            for j, s in enumerate(("k", "q")):
                pn = psnk if s == "k" else psnq
                for h in range(HL):
                    sq = sqp.tile([128, LT], BF16, name="sq", tag="sq")
                    nc.scalar.square(sq, raw[(s, h)])
                    nc.tensor.matmul(pn, oh8[:, h, 0:4], sq,
                                     start=(h == 0), stop=(h == HL - 1))
            # rk = 1/(sqrt(nk2)+eps); rkb = rk*bp; qn2e = eps*nq2
            nrm = bchp.tile([4, LT], F32, name="nrm", tag="bchain")
            nc.scalar.sqrt(nrm, psnk)
            nrme = bchp.tile([4, LT], F32, name="nrme", tag="bchain")
            nc.vector.tensor_scalar_add(nrme, nrm, EPS)
            nc.vector.reciprocal_approx_fast(rk_cm[:, tsl], nrme)
            nc.vector.tensor_mul(rkb_cm[:, tsl], rk_cm[:, tsl], bp)
            nc.vector.tensor_scalar_mul(qn2e_cm[:, tsl], psnq, EPS)
            # transpose scale rows for each chunk of this lt -> [128, 12]
            bt = btp.tile([128, 4, 12], F32, name="bt", tag="bt")
            for cc in range(4):
                cidx = lt * 4 + cc
                csl = bass.ds(cidx * C, C)
                pb = ps_t.tile([128, 12], F32, name="psbt", tag="pst")
                nc.tensor.transpose(pb[:, 0:4], rk_cm[:, csl], id32[0:4, 0:4])
                nc.tensor.transpose(pb[:, 4:8], rkb_cm[:, csl], id32[0:4, 0:4])
                nc.tensor.transpose(pb[:, 8:12], qn2e_cm[:, csl], id32[0:4, 0:4])
                nc.vector.tensor_copy(bt[:, cc, :], pb)
                bt_tiles[cidx] = bt[:, cc, :]

        def emit_chunk_a1(cidx, raw):
            """transposes + scale folds + first mask matmuls for chunk cidx."""
            cc = cidx % 4
            csl = bass.ds(cc * C, C)
            bt = bt_tiles[cidx]

            # [silu(v) | k] transposed to token-major
            pkv = {}
            for h in range(HL):
                p = ps_t.tile([128, 256], BF16, name="pkv", tag="pst")
                nc.tensor.transpose(p[:, 0:128], raw[("v", h)][:, csl], id16)
                nc.tensor.transpose(p[:, 128:256], raw[("k", h)][:, csl], id16)
                pkv[h] = p
            kntm, x0 = {}, {}
            for h in range(HL):
                kt = normp.tile([128, 128], BF16, name="kntm", tag=f"kt{h}")
                nc.vector.tensor_scalar_mul(kt, pkv[h][:, 128:256], bt[:, 0 + h:1 + h])
                kntm[h] = kt
                x = normp.tile([128, 256], BF16, name="x0", tag=f"x{h}", bufs=4)
                nc.vector.tensor_scalar_mul(x, pkv[h], bt[:, 4 + h:5 + h])
                x0[h] = x
            kpt = {}
            for h in range(HL):
                p3 = ps_t.tile([128, 128], BF16, name="ps3", tag="pst")
                nc.tensor.transpose(p3, x0[h][:, 128:256], id16)
                kp = normp.tile([128, 128], BF16, name="kpt", tag=f"kp{h}")
                nc.vector.tensor_copy(kp, p3)
                kpt[h] = kp
            lo, nt = {}, {}
            for h in range(HL):
                psa = ps_mm.tile([128, 256], F32, name="psa", tag="mm")
                nc.tensor.matmul(psa[:, 0:128], raw[("k", h)][:, csl],
                                 raw[("q", h)][:, csl])
                nc.tensor.matmul(psa[:, 128:256], raw[("k", h)][:, csl], kpt[h])
                lo_t = normp.tile([128, 128], BF16, name="lo", tag=f"lo{h}")
                nc.vector.scalar_tensor_tensor(lo_t, psa[:, 0:128], bt[:, 0 + h:1 + h],
                                               mk[:, 1, :], op0=ALU.mult, op1=ALU.mult)
                lo[h] = lo_t
                nt_t = chainp.tile([128, 128], BF16, name="ntl", tag="ntl")
                nc.vector.scalar_tensor_tensor(nt_t, psa[:, 128:256], bt[:, 0 + h:1 + h],
                                               mk[:, 0, :], op0=ALU.mult, op1=ALU.mult)
                nt[h] = nt_t
            return dict(cidx=cidx, csl=csl, bt=bt, x0=x0, lo=lo, nt=nt,
                        kntm=kntm, raw=raw)

        def emit_chunk_a2(st):
            """Minv chain + applies + ukt for chunk cidx."""
            nt, x0 = st["nt"], st["x0"]
            nm = {}
            for h in range(HL):
                pnm = ps_t.tile([128, 128], BF16, name="pnm", tag="pst")
                nc.tensor.transpose(pnm, nt[h], id16)
                nm_t = chainp.tile([128, 128], BF16, name="nml", tag="nml")
                nc.vector.tensor_copy(nm_t, pnm)
                nm[h] = nm_t

            def mm_copy(lhsT, rhs, name):
                p = ps_mm.tile([128, rhs.shape[-1]], F32, name="psc", tag="mm")
                nc.tensor.matmul(p, lhsT, rhs)
                t = chainp.tile([128, rhs.shape[-1]], BF16, name=name, tag=name)
                nc.vector.tensor_copy(t, p)
                return t

            p1, t1, t2 = {}, {}, {}
            for h in range(HL):
                p1[h] = mm_copy(nt[h], nm[h], "cp1")
            for h in range(HL):
                t1[h] = mm_copy(nm[h], nt[h], "ct1")
            for h in range(HL):
                t2[h] = mm_copy(p1[h], t1[h], "ct2")

            def apply_step(lhs, xin, h, sub=False):
                p = ps_mm.tile([128, 256], F32, name="psx", tag="mm")
                nc.tensor.matmul(p, lhs, xin)
                xo = normp.tile([128, 256], BF16, name="xs", tag=f"x{h}", bufs=4)
                if sub:
                    nc.vector.tensor_sub(xo, xin, p)
                else:
                    nc.vector.tensor_add(xo, xin, p)
                return xo

            xs = dict(x0)
            for h in range(HL):
                xs[h] = apply_step(nt[h], xs[h], h, sub=True)
            for lev in (t1, t2):
                for h in range(HL):
                    xs[h] = apply_step(lev[h], xs[h], h)
            ukt = {}
            for h in range(HL):
                p = ps_t.tile([128, 128], BF16, name="psukt", tag="pst")
                nc.tensor.transpose(p, xs[h][:, 128:256], id16)
                t = chainp.tile([128, 128], BF16, name="ukt", tag="ukt")
                nc.vector.tensor_copy(t, p)
                ukt[h] = t
            st["xs"] = xs
            st["ukt"] = ukt

        def emit_chunk_bstate(st):
            """pu/u/pd/po matmuls + state update; po parked in SBUF (pob)."""
            cidx, csl, xs, lo, kntm, ukt, raw = (
                st["cidx"], st["csl"], st["xs"], st["lo"],
                st["kntm"], st["ukt"], st["raw"])
            u = {}
            pus = {}
            for h in range(HL):
                pu = ps_mm.tile([128, 128], F32, name="psu", tag="mm")
                nc.tensor.matmul(pu, ukt[h], s16[:, h, :])
                pus[h] = pu
            for h in range(HL):
                ut = chainp.tile([128, 128], BF16, name="u", tag="u")
                nc.vector.tensor_sub(ut, xs[h][:, 0:128], pus[h])
                u[h] = ut
            # state chain first: next chunk's pu/po wait only on this
            pds = {}
            for h in range(HL):
                pd = ps_mm.tile([128, 128], F32, name="psd", tag="mm")
                nc.tensor.matmul(pd, kntm[h], u[h])
                pds[h] = pd
            pos = {}
            for h in range(HL):
                po = ps_mm.tile([128, 128], F32, name="pso", tag="mm")
                nc.tensor.matmul(po, lo[h], u[h], start=True, stop=False)
                nc.tensor.matmul(po, raw[("q", h)][:, csl], s16[:, h, :],
                                 start=False, stop=True)
                pos[h] = po
            pob = {}
            for h in range(HL):
                pb_ = chainp.tile([128, 128], BF16, name="pob", tag="pob")
                nc.vector.tensor_copy(pb_, pos[h])
                pob[h] = pb_
            for h in range(HL):
                nc.vector.tensor_add(s32[:, h, :], s32[:, h, :], pds[h])
                nc.vector.tensor_copy(s16[:, h, :], s32[:, h, :])
            ms4 = scalp.tile([128, 4], F32, name="ms4")
            for h in range(HL):
                junk = scalp.tile([128, 128], F32, name="junk", tag="junk", bufs=2)
                nc.scalar.activation(junk, pob[h], AF.Square,
                                     accum_out=ms4[:, h:h + 1])
            st["pob"] = pob
            st["ms4"] = ms4

        def emit_chunk_bout(st):
            """RMS scale + output transpose; deferred off the PE critical path."""
            cidx, bt, pob, ms4 = st["cidx"], st["bt"], st["pob"], st["ms4"]
            gsl = bass.ds(cidx * C, C)
            tt4 = scalp.tile([128, 4], F32, name="tt4")
            nc.vector.scalar_tensor_tensor(tt4, ms4, 1.0 / HD, bt[:, 8:12],
                                           op0=ALU.mult, op1=ALU.add)
            sq4 = scalp.tile([128, 4], F32, name="sq4")
            nc.scalar.sqrt(sq4, tt4)
            ro4 = scalp.tile([128, 4], F32, name="ro4")
            nc.vector.reciprocal_approx_fast(ro4, sq4)
            for h in range(HL):
                onb = chainp.tile([128, 128], BF16, name="onb", tag="onb")
                nc.vector.tensor_scalar_mul(onb, pob[h], ro4[:, h:h + 1])
                pot = ps_t.tile([128, 128], BF16, name="psot", tag="pst")
                nc.tensor.transpose(pot, onb, id16)
                nc.vector.tensor_copy(ot[:, h, gsl], pot)

        def emit_outproj_tile(cidx):
            tok = bass.ds(cidx * 128, 128)
            for oc in range(2):
                p = ps_big.tile([128, 512], F32, name="psop", tag="big")
                for h in range(HL):
                    nc.tensor.matmul(p, ot[:, h, tok],
                                     wo[:, h, oc * 512:(oc + 1) * 512],
                                     start=(h == 0), stop=(h == HL - 1))
                st = outp.tile([128, 512], F32, name="ost", tag="ost")
                nc.scalar.copy(st, p)
                nc.sync.dma_start(out=out_d[tok, oc * 512:(oc + 1) * 512], in_=st)

        # ---- schedule ----
        emit_projconv(0, 0)
        emit_projconv(0, 1)
        emit_norms(0)
        pending = None
        for lt in range(NLT):
            raw = raw_cur[lt]
            for cc in range(4):
                cidx = lt * 4 + cc
                sta = emit_chunk_a1(cidx, raw)
                if pending is not None:
                    emit_chunk_bstate(pending)
                emit_chunk_a2(sta)
                if pending is not None:
                    emit_chunk_bout(pending)
                    emit_outproj_tile(pending["cidx"])
                pending = sta
                if lt < NLT - 1:
                    if cc == 0:
                        emit_projconv(lt + 1, 0)
                    elif cc == 1:
                        emit_projconv(lt + 1, 1)
                    elif cc == 2:
                        emit_norms(lt + 1)
        emit_chunk_bstate(pending)
        emit_chunk_bout(pending)
        emit_outproj_tile(pending["cidx"])

    nc.compile()
    return nc


# ---------------- host side ----------------

def _bf(x):
    return np.ascontiguousarray(np.asarray(x, np.float32)).astype(ml_dtypes.bfloat16)


def host_prep(inputs):
    x = np.asarray(inputs["x"], np.float32)
    rms_vec = np.tile(np.asarray(inputs["rms_w"], np.float32), H)
    wo_eff = np.asarray(inputs["Wo"], np.float32) * rms_vec[None, :]

    masks = np.stack([
        np.triu(np.ones((128, 128), np.float32), 1),
        np.triu(np.ones((128, 128), np.float32), 0),
    ]).astype(np.float32)
    ident = np.eye(128, dtype=np.float32)
    oneh = np.zeros((8, 128, 8), np.float32)
    for j in range(8):
        oneh[j, :, j] = 1.0

    for nm in ("bq", "bk", "bv", "bbeta", "bo", "convb_q", "convb_k", "convb_v"):
        assert np.all(np.asarray(inputs[nm]) == 0.0), f"nonzero bias {nm} unsupported"

    in_maps = []
    for c in range(8):
        b, hh = c // 2, c % 2
        rows = slice(hh * 512, (hh + 1) * 512)
        cds = []
        for s in ("k", "q", "v"):
            cwf = np.asarray(inputs[f"conv_{s}"], np.float32)[rows]
            for h in range(HL):
                cds.append(np.stack([np.diag(cwf[h * 128:(h + 1) * 128, j])
                                     for j in range(CONV)]))
        m = {
            "xt": _bf(x[b].T.reshape(KS, 128, L)),
            "wq": _bf(np.asarray(inputs["Wq"], np.float32)[rows].T.reshape(KS, 128, 512)),
            "wk": _bf(np.asarray(inputs["Wk"], np.float32)[rows].T.reshape(KS, 128, 512)),
            "wv": _bf(np.asarray(inputs["Wv"], np.float32)[rows].T.reshape(KS, 128, 512)),
            "wb": _bf(np.asarray(inputs["Wbeta"], np.float32)[hh * 4:(hh + 1) * 4].T.reshape(KS, 128, 4)),
            "wo": _bf(wo_eff[:, rows].T.reshape(4, 128, 1024)),
            "cd": np.stack(cds).astype(ml_dtypes.bfloat16),
            "mk": masks,
            "oh": _bf(oneh),
            "id16": _bf(ident),
            "id32": ident,
        }
        in_maps.append(m)
    return in_maps


def host_combine(results, inputs):
    bo = np.asarray(inputs["bo"], np.float32)
    out = np.zeros((B, L, D), np.float32)
    for b in range(B):
        out[b] = results[2 * b]["out"] + results[2 * b + 1]["out"] + bo
    return out


# ---------------- entry point ----------------

_NC_CACHE = []


def kernel(**inputs):
    """Full-input DeltaNet layer distributed over 8 NeuronCores.

    Shards batch (4) x head-group (2) across cores, runs the Bass kernel via
    run_bass_kernel_spmd, and reduces the per-pair partial out-projections on
    the host (the pair all-reduce) before returning [4, 2048, 1024] fp32.
    """
    from concourse.bass_utils import run_bass_kernel_spmd

    if not _NC_CACHE:
        _NC_CACHE.append(build_nc())
    nc = _NC_CACHE[0]
    in_maps = host_prep(inputs)
    br = run_bass_kernel_spmd(nc, in_maps, list(range(8)))
    return host_combine(br.results, inputs)
